# revision 37
# baseline (speedup 1.0000x reference)
"""Multi-head self-attention on 8 Trainium2 NeuronCores.

Problem: x[2, 2048, 1024], 16 heads, Dh=64, fp32.
  q/k/v = x @ W.T ; scores = q k^T / 8 ; out = softmax(scores) v @ W_o.T

Sharding (Megatron-style): each core owns 2 heads (128 of the 1024 model
dims). W_q/W_k/W_v column-sharded, W_o row-sharded; the cross-core
all-reduce of the output-projection partials is done on the host.

Per-core layout strategy:
  - x is fed pre-transposed (xT [1024, 4096]) so all matmuls contract on
    the partition axis with fully contiguous DMA.
  - qT/kT computed as [128 = 2 heads x 64, 4096] (dims on partitions).
  - Scores computed transposed, S^T[k_tok, q_tok], heads packed in PE
    row-groups (K=64 each at partition bases 0/64) -> one [128, 1024]
    2-bank PSUM tile per k-strip; exp() on ScalarE reads it directly.
  - v in natural layout via PE transpose, augmented with a ones column:
    PV matmul (M=65) yields both the weighted values and the softmax
    denominators in one accumulation chain.
  - Denominators gathered via DMA partition-remap, reciprocal on VectorE,
    broadcast via K=1 outer-product matmuls, applied as one
    tensor_tensor multiply.
  - All big matmuls run as float32r (fp22 mantissa truncation, 4x faster
    than true fp32 on the PE).
"""

import os
from contextlib import ExitStack

import numpy as np

import concourse.bass as bass
import concourse.tile as tile
from concourse import bacc, mybir
from concourse._compat import with_exitstack
from concourse.bass_utils import run_bass_kernel_spmd

F32 = mybir.dt.float32
F32R = mybir.dt.float32r
EXP = mybir.ActivationFunctionType.Exp

P = 128          # partitions / head-pair dims per core
D = 1024         # model dim
T = 2048         # tokens per batch
NB = 2           # batches
BT = NB * T      # 4096 flattened tokens
KT = D // P      # 8 contraction tiles over model dim
NQ = T // 512    # 4 q-tiles of 512 per batch
NS = T // P      # 16 k-strips of 128 per batch
N_CORES = 8


@with_exitstack
def _mhsa_kernel(ctx: ExitStack, tc: tile.TileContext, out, xT, wqT, wkT, wvT,
                 woT, ident_in, ones_in):
    nc = tc.nc

    # ---- pools ----
    wpool = ctx.enter_context(tc.tile_pool(name="weights", bufs=1))
    xpool = ctx.enter_context(tc.tile_pool(name="xtiles", bufs=16))
    qkpool = ctx.enter_context(tc.tile_pool(name="qk", bufs=1))
    vtpool = ctx.enter_context(tc.tile_pool(name="vt", bufs=2))
    vapool = ctx.enter_context(tc.tile_pool(name="vaug", bufs=1))
    expool = ctx.enter_context(tc.tile_pool(name="expp", bufs=6))
    aupool = ctx.enter_context(tc.tile_pool(name="avun", bufs=5))
    dpool = ctx.enter_context(tc.tile_pool(name="dd", bufs=1))
    anpool = ctx.enter_context(tc.tile_pool(name="avnorm", bufs=3))
    opool = ctx.enter_context(tc.tile_pool(name="outsb", bufs=4))
    rpool = ctx.enter_context(tc.tile_pool(name="recip", bufs=2))
    drpool = ctx.enter_context(tc.tile_pool(name="dscr", bufs=4, space="DRAM"))

    ps_sc = ctx.enter_context(tc.tile_pool(name="ps_sc", bufs=2, space="PSUM"))
    ps_pv = ctx.enter_context(tc.tile_pool(name="ps_pv", bufs=2, space="PSUM"))
    ps_misc = ctx.enter_context(tc.tile_pool(name="ps_misc", bufs=2, space="PSUM"))

    # ---- weights / constants (resident) ----
    wq_sb = [wpool.tile([P, P], F32R, name=f"wq{k}") for k in range(KT)]
    wk_sb = [wpool.tile([P, P], F32R, name=f"wk{k}") for k in range(KT)]
    wv_sb = [wpool.tile([P, P], F32R, name=f"wv{k}") for k in range(KT)]
    # wq first (the first QKV chain needs all 8 of them), split across the
    # sync and (startup-idle) scalar DMA queues to halve issue latency.
    for k in range(KT):
        eng = nc.sync if k % 2 == 0 else nc.scalar
        eng.dma_start(out=wq_sb[k][:], in_=wqT[k * P:(k + 1) * P, :])
    for k in range(KT):
        eng = nc.sync if k % 2 == 0 else nc.scalar
        eng.dma_start(out=wk_sb[k][:], in_=wkT[k * P:(k + 1) * P, :])
    for k in range(KT):
        eng = nc.sync if k % 2 == 0 else nc.scalar
        eng.dma_start(out=wv_sb[k][:], in_=wvT[k * P:(k + 1) * P, :])
    wo_sb = wpool.tile([P, D], F32R, name="wo_sb")
    nc.sync.dma_start(out=wo_sb[:], in_=woT[:])
    ident = wpool.tile([P, P], F32, name="ident")
    nc.scalar.dma_start(out=ident[:], in_=ident_in[:])

    qT = qkpool.tile([P, BT], F32R, name="qT")
    kTt = qkpool.tile([P, BT], F32R, name="kTt")

    # D-row staging: softmax denominators live on partition 64 only.
    dst = dpool.tile([65, 2 * T], F32R, name="dst")

    va = {}        # (b, 'A'/'B') -> augmented-v tile
    av_uns = {}    # (b, n) -> unnormalized attention output tile
    rscrs = {}     # b -> reciprocal DRAM scratch

    def emit_qkv_chunks(b):
        """Closure list emitting batch-b QKV projections + v transpose.

        Split into small chunks so they can be interleaved into the other
        batch's (ScalarE-paced) attention loop as PE filler work.
        """
        chunks = []
        xts_by_n = {}

        def init():
            vA = vapool.tile([P, NS * 65], F32R, name=f"vA{b}")
            vB = vapool.tile([P, NS * 65], F32R, name=f"vB{b}")
            nc.gpsimd.dma_start(out=vA[:], in_=ones_in[:])
            nc.gpsimd.dma_start(out=vB[:], in_=ones_in[:])
            va[(b, "A")] = vA
            va[(b, "B")] = vB
            vT = vtpool.tile([P, T], F32, name="vT", tag="vt")
            va[(b, "T")] = vT
        chunks.append(init)

        def load_n(n):
            def f():
                col = b * T + n * 512
                xts = []
                for k in range(KT):
                    xt = xpool.tile([P, 512], F32R, name=f"xt{b}_{n}_{k}",
                                    tag="xt")
                    nc.gpsimd.dma_start(
                        out=xt[:], in_=xT[k * P:(k + 1) * P, col:col + 512])
                    xts.append(xt)
                xts_by_n[n] = xts
            return f

        def chain_parts(n, which):
            """The 8-matmul accumulation split into two filler chunks so a
            single drain never blocks ScalarE for a whole 1.9us chain."""
            cell = {}

            def f1():
                acc = ps_misc.tile([P, 512], F32, name="qkv_ps", tag="mps")
                cell["acc"] = acc
                w_sb = {"q": wq_sb, "k": wk_sb, "v": wv_sb}[which]
                for k in range(KT // 2):
                    nc.tensor.matmul(
                        acc[:], w_sb[k], xts_by_n[n][k],
                        start=(k == 0), stop=False,
                    )

            def f2():
                col = b * T + n * 512
                w_sb, dst_tile, dcol = {
                    "q": (wq_sb, qT, col),
                    "k": (wk_sb, kTt, col),
                    "v": (wv_sb, va[(b, "T")], n * 512),
                }[which]
                acc = cell["acc"]
                for k in range(KT // 2, KT):
                    nc.tensor.matmul(
                        acc[:], w_sb[k], xts_by_n[n][k],
                        start=False, stop=(k == KT - 1),
                    )
                nc.vector.tensor_copy(dst_tile[:, dcol:dcol + 512], acc[:])
            return f1, f2

        def trans(s):
            def f():
                vT = va[(b, "T")]
                tp = ps_misc.tile([P, P], F32, name="tr_ps", tag="mps")
                nc.tensor.transpose(tp[:], vT[:, s * P:(s + 1) * P], ident[:])
                nc.vector.tensor_copy(
                    va[(b, "A")][:, s * 65:s * 65 + 64], tp[:, 0:64])
                nc.vector.tensor_copy(
                    va[(b, "B")][:, s * 65:s * 65 + 64], tp[:, 64:128])
            return f

        for n in range(NQ):
            chunks.append(load_n(n))
            for which in ("q", "k", "v"):
                chunks.extend(chain_parts(n, which))
            for s in range(4 * n, 4 * n + 4):
                chunks.append(trans(s))
        return chunks

    def emit_recip_n(b, n):
        """Per-q-tile denominator reciprocal: gather, invert, scatter."""
        rin = rpool.tile([P, 8], F32R, name="rin", tag="rin")
        rout = rpool.tile([P, 8], F32R, name="rout", tag="rout")
        dscr = drpool.tile([2, 512], F32R, name="dscr", tag="dscr")
        rscr = drpool.tile([2, 512], F32R, name="rscr", tag="rscr")
        nc.sync.dma_start(out=dscr[0:1, :],
                          in_=dst[64:65, n * 512:(n + 1) * 512])
        nc.sync.dma_start(out=dscr[1:2, :],
                          in_=dst[64:65, T + n * 512:T + (n + 1) * 512])
        nc.sync.dma_start(
            out=rin[:], in_=dscr.rearrange("p (a c) -> (p a) c", a=64))
        with nc.allow_low_precision(reason="fp22 softmax denominators"):
            nc.vector.reciprocal(rout[:], rin[:])
        nc.sync.dma_start(
            out=rscr.rearrange("p (a c) -> (p a) c", a=64), in_=rout[:])
        rscrs[(b, n)] = rscr

    def make_norm_outproj(b):
        """Per-q-tile closure lists: normalize + output projection."""
        avns = {}

        def norm(n):
            def f():
                rscr = rscrs[(b, n)]
                bc_sb = anpool.tile([P, 512], F32R, name="bc_sb", tag="bcs")
                nc.gpsimd.dma_start(
                    out=bc_sb[0:64, :],
                    in_=rscr[0:1, :].to_broadcast((64, 512)),
                )
                nc.gpsimd.dma_start(
                    out=bc_sb[64:128, :],
                    in_=rscr[1:2, :].to_broadcast((64, 512)),
                )
                av_n = anpool.tile([P, 512], F32R, name="av_n", tag="avn")
                nc.vector.tensor_mul(av_n[:], av_uns[(b, n)][:], bc_sb[:])
                avns[n] = av_n
            return f

        def outproj(n, sub):
            def f():
                av_n = avns[n]
                row0 = b * T + n * 512 + sub * P
                for jh in range(2):
                    op = ps_misc.tile([P, 512], F32, name="op_ps", tag="mps")
                    nc.tensor.matmul(
                        op[:],
                        av_n[:, sub * P:(sub + 1) * P],
                        wo_sb[:, jh * 512:(jh + 1) * 512],
                        start=True, stop=True,
                    )
                    ot = opool.tile([P, 512], F32, name="ot", tag="ot")
                    nc.vector.tensor_copy(ot[:], op[:])
                    nc.gpsimd.dma_start(
                        out=out[row0:row0 + P, jh * 512:(jh + 1) * 512],
                        in_=ot[:],
                    )
            return f

        def per_n(n):
            return [norm(n)] + [outproj(n, sub) for sub in range(4)]
        return per_n

    def attn_batch(b, fillers, self_np=None):
        """Batch-b attention; filler chunks drained between strip steps."""
        from collections import deque
        queue = deque(fillers)
        for n in range(NQ):
            qcol = b * T + n * 512
            avA = ps_pv.tile([P, 512], F32, name="avA", tag="pv")
            avB = ps_pv.tile([P, 512], F32, name="avB", tag="pv")
            for s in range(NS):
                kcol = b * T + s * P
                sc = ps_sc.tile([P, 1024], F32, name="sc", tag="sc")
                nc.tensor.matmul(
                    sc[:, 0:512],
                    kTt[0:64, kcol:kcol + P],
                    qT[0:64, qcol:qcol + 512],
                    start=True, stop=True,
                )
                nc.tensor.matmul(
                    sc[:, 512:1024],
                    kTt[64:128, kcol:kcol + P],
                    qT[64:128, qcol:qcol + 512],
                    start=True, stop=True,
                )
                ex = expool.tile([P, 1024], F32R, name="ex", tag="ex")
                nc.scalar.activation(out=ex[:], in_=sc[:], func=EXP,
                                     scale=0.125)
                # PE filler work goes here, during the exp wait.
                if queue:
                    queue.popleft()()
                nc.tensor.matmul(
                    avA[0:65, :],
                    va[(b, "A")][:, s * 65:(s + 1) * 65],
                    ex[:, 0:512],
                    start=(s == 0), stop=(s == NS - 1),
                )
                nc.tensor.matmul(
                    avB[0:65, :],
                    va[(b, "B")][:, s * 65:(s + 1) * 65],
                    ex[:, 512:1024],
                    start=(s == 0), stop=(s == NS - 1),
                )
            av_un = aupool.tile([P, 512], F32, name="av_un", tag="avun")
            nc.vector.tensor_copy(av_un[0:64, :], avA[0:64, :])
            nc.vector.tensor_copy(av_un[64:128, :], avB[0:64, :])
            nc.vector.tensor_copy(dst[64:65, n * 512:(n + 1) * 512],
                                  avA[64:65, :])
            nc.vector.tensor_copy(dst[64:65, T + n * 512:T + (n + 1) * 512],
                                  avB[64:65, :])
            av_uns[(b, n)] = av_un
            emit_recip_n(b, n)
            if self_np is not None:
                queue.extend(self_np(n))
        while queue:
            queue.popleft()()

    # ---- software-pipelined schedule ----
    for c in emit_qkv_chunks(0):
        c()
    attn_batch(0, emit_qkv_chunks(1))
    np0 = make_norm_outproj(0)
    np1 = make_norm_outproj(1)
    fillers0 = [c for n in range(NQ) for c in np0(n)]
    attn_batch(1, fillers0, self_np=np1)


_PROGRAM = None


def _build_program():
    nc = bacc.Bacc(
        "TRN2", target_bir_lowering=False, debug=False,
        enable_asserts=False, num_devices=N_CORES,
    )
    xT = nc.dram_tensor("xT", [D, BT], F32R, kind="ExternalInput").ap()
    wqT = nc.dram_tensor("wqT", [D, P], F32R, kind="ExternalInput").ap()
    wkT = nc.dram_tensor("wkT", [D, P], F32R, kind="ExternalInput").ap()
    wvT = nc.dram_tensor("wvT", [D, P], F32R, kind="ExternalInput").ap()
    woT = nc.dram_tensor("woT", [P, D], F32R, kind="ExternalInput").ap()
    ident_in = nc.dram_tensor("ident_in", [P, P], F32, kind="ExternalInput").ap()
    ones_in = nc.dram_tensor("ones_in", [P, NS * 65], F32R,
                             kind="ExternalInput").ap()
    out = nc.dram_tensor("out", [BT, D], F32, kind="ExternalOutput").ap()
    with tile.TileContext(nc) as tc:
        _mhsa_kernel(tc, out, xT, wqT, wkT, wvT, woT, ident_in, ones_in)
    nc.compile()
    return nc


def get_program():
    global _PROGRAM
    if _PROGRAM is None:
        _PROGRAM = _build_program()
    return _PROGRAM


last_results = None


def _install_trace_hook():
    """Register the axon NTFF-profile hook that the agent image's antenv
    lacks, so run_bass_kernel_spmd(trace=True) can capture HW timings."""
    import sys
    import types

    if "antenv.axon_hooks" in sys.modules:
        return
    try:
        from trn_agent_boot.trn_boot import _ntff_profile_via_ctypes
        hook = _ntff_profile_via_ctypes("/opt/axon/libaxon_pjrt.so")
    except Exception:
        hook = None
    mod = types.ModuleType("antenv.axon_hooks")
    state = {"hook": hook}
    mod.get_axon_ntff_profile_hook = lambda: state["hook"]
    mod.set_axon_ntff_profile_hook = lambda h: state.__setitem__("hook", h)
    sys.modules["antenv.axon_hooks"] = mod

    import concourse.bass_utils as bu
    orig_upload = bu.upload_artifacts

    def safe_upload(tmpdir):
        try:
            return orig_upload(tmpdir)
        except Exception:
            return tmpdir

    bu.upload_artifacts = safe_upload


def kernel(x, W_q, W_k, W_v, W_o):
    global last_results
    x = np.ascontiguousarray(np.asarray(x, dtype=np.float32))
    W_q = np.asarray(W_q, dtype=np.float32)
    W_k = np.asarray(W_k, dtype=np.float32)
    W_v = np.asarray(W_v, dtype=np.float32)
    W_o = np.asarray(W_o, dtype=np.float32)

    xTn = np.ascontiguousarray(x.reshape(BT, D).T)
    ident = np.eye(P, dtype=np.float32)
    ones_arr = np.ones((P, NS * 65), dtype=np.float32)
    in_maps = []
    for c in range(N_CORES):
        sl = slice(P * c, P * (c + 1))
        in_maps.append({
            "xT": xTn,
            "wqT": np.ascontiguousarray(W_q[sl, :].T),
            "wkT": np.ascontiguousarray(W_k[sl, :].T),
            "wvT": np.ascontiguousarray(W_v[sl, :].T),
            "woT": np.ascontiguousarray(W_o[:, sl].T),
            "ident_in": ident,
            "ones_in": ones_arr,
        })

    trace = bool(int(os.environ.get("KERNEL_TRACE", "0")))
    if trace:
        _install_trace_hook()
    nc = get_program()
    res = run_bass_kernel_spmd(
        nc, in_maps, core_ids=list(range(N_CORES)), trace=trace,
    )
    last_results = res
    total = res.results[0]["out"].astype(np.float32)
    for r in res.results[1:]:
        total = total + r["out"]
    return total.reshape(NB, T, D)
